# revision 20
# baseline (speedup 1.0000x reference)
"""Self-contained Trainium2 Bass kernel for the 5-layer GraphConv GNN
(N=100000 nodes, E=3200000 edges, dims 6->20->15->10->5->2, relu, softmax).

kernel(**inputs) takes the full unsharded inputs (as from setup_inputs()),
shards edges across 8 NeuronCores by destination-node range internally,
runs the Bass program via run_bass_kernel_spmd, and returns the full
[100000, 2] float32 output.

Key design points (all measured on HW):
- Per-edge gathers of table rows y=x@W_rel (padded to 256B) via SWDGE
  dma_gather, spread over 4 SWDGE queues (the single queue serializes
  descriptor processing, 2.2x).
- Edges sorted by (dst-block, src-chunk, src); exact per-group counts are
  baked as num_idxs_reg so padding slots cost no DMA descriptors.
- Source nodes are chunked in a core-interleaved layout so the layer
  boundary AllGather splits into NCHUNK independent chunk collectives,
  each issued as soon as its ybounce rows are ready -- they overlap the
  tail of the previous layer's gather/matmul stream.
- Segment-sum per dst block via one-hot (is_equal vs iota) matmul into
  PSUM, accumulated across chunks in SBUF.
"""

import sys
sys.path.insert(0, '/opt/trn_rl_repo')
import numpy as np
import concourse.bass as bass
import concourse.bacc as bacc
import concourse.tile as tile
from concourse import mybir

f32 = mybir.dt.float32
f16 = mybir.dt.float16
i32 = mybir.dt.int32
i16 = mybir.dt.int16

NCHUNK = 5
PAD = 64  # table row padded to 64 f32 = 256 bytes (dma_gather elem quantum)

DIMS = [6, 20, 15, 10, 5, 2]
N_NODES = 100000
N_CORES = 8


def preprocess(edge_index, edge_weight, N, ncores):
    """Group edges by (core, src-chunk, dst-block); sort by src within group.

    Chunks are core-interleaved: chunk q covers per-core row slice
    [q*CPC, (q+1)*CPC) of every core, so the chunk table (and its
    AllGather) only needs those rows. Chunk-table row for global src s:
    (s // NPC) * CPC + (s % NPC) - q * CPC  where q = (s % NPC) // CPC.

    Slot layout per (q, b) group: [0:cnt) real edges, [cnt:gmax) dummy
    idx 0 / w 0 (gmax = max count over cores -> uniform num_idxs_reg),
    [gmax:SP) idx -1 (descriptors skipped).
    """
    src = np.asarray(edge_index[0], dtype=np.int64)
    dst = np.asarray(edge_index[1], dtype=np.int64)
    w = np.asarray(edge_weight, dtype=np.float32)
    NPC = N // ncores
    NB = (NPC + 127) // 128
    last_cnt = NPC - (NB - 1) * 128
    CPC = NPC // NCHUNK               # per-core rows per chunk
    CH = CPC * ncores                 # rows per chunk table

    core = dst // NPC
    r = dst % NPC
    blk = r // 128
    loc = r % 128
    sq = (src % NPC) // CPC
    srcl = (src // NPC) * CPC + (src % NPC) - sq * CPC
    key = (core * NCHUNK + sq) * NB + blk
    order = np.lexsort((src, key))
    srcl_s, loc_s, w_s, key_s = srcl[order], loc[order], w[order], key[order]

    ngroups = ncores * NCHUNK * NB
    counts = np.bincount(key_s, minlength=ngroups)
    T_chunk = max(1, int(np.ceil(counts.max() / 128)))
    SP = T_chunk * 128

    srcl_p = np.full((ngroups, SP), -1, np.int16)
    loc_p = np.zeros((ngroups, SP), np.float16)
    w_p = np.zeros((ngroups, SP), np.float32)
    starts = np.concatenate([[0], np.cumsum(counts)[:-1]])
    pos = np.arange(len(key_s)) - starts[key_s]
    srcl_p[key_s, pos] = srcl_s
    loc_p[key_s, pos] = loc_s
    w_p[key_s, pos] = w_s
    gcounts = counts.reshape(ncores, NCHUNK * NB).max(axis=0)
    for k in range(ncores):
        for g in range(NCHUNK * NB):
            row = k * NCHUNK * NB + g
            srcl_p[row, counts[row]:gcounts[g]] = 0

    # idx16: per (q, b) group, slot j -> [j % 16, j // 16], replicated x8
    idx_wrap = np.ascontiguousarray(
        srcl_p.reshape(ngroups, SP // 16, 16).transpose(0, 2, 1))
    idx_wrap = np.tile(idx_wrap, (1, 8, 1))  # [g, 128, SP/16]
    idx16 = idx_wrap.reshape(ncores, NCHUNK * NB, 128, SP // 16)

    def to_sbuf(a):
        # [ngroups, SP] -> [ncores, 128, NCHUNK*NB*T_chunk]
        # column (q*NB + b)*T_chunk + t holds slot t*128+p at partition p
        a = a.reshape(ncores, NCHUNK * NB, T_chunk, 128)
        return np.ascontiguousarray(
            a.transpose(0, 3, 1, 2).reshape(ncores, 128, NCHUNK * NB * T_chunk))

    gcounts = gcounts.reshape(NCHUNK, NB)
    return (idx16, to_sbuf(loc_p), to_sbuf(w_p), T_chunk, NB, NPC, last_cnt,
            CPC, CH, gcounts)


def build_gnn(nc, N, NPC, NB, T_chunk, dims, ncores, last_cnt, CPC, CH,
              gcounts, debug=False):
    """Emit the full 5-layer program into nc (chunk-major block loop)."""
    L = len(dims) - 1
    SP = T_chunk * 128
    IC = SP // 16                 # idx columns per (q, b) group
    NQ = nc.num_swdge_queues
    qctr = [0]
    cmax = dims[1]

    # ---- DRAM I/O ----
    idx_d = nc.dram_tensor("idx16", [NCHUNK * NB, 128, IC], i16, kind="ExternalInput")
    dstl_d = nc.dram_tensor("dstl", [128, NCHUNK * NB * T_chunk], f16, kind="ExternalInput")
    wgt_d = nc.dram_tensor("wgt", [128, NCHUNK * NB * T_chunk], f32, kind="ExternalInput")
    xT0_d = nc.dram_tensor("xT0", [dims[0], NPC], f32, kind="ExternalInput")
    x64_d = nc.dram_tensor("x64", [N, PAD], f32, kind="ExternalInput")
    iota_d = nc.dram_tensor("iota", [128, 128], f16, kind="ExternalInput")
    ident_d = nc.dram_tensor("ident", [128, 128], f32, kind="ExternalInput")
    wrel_d = [nc.dram_tensor(f"wrel{l}", [dims[l], dims[l + 1]], f32, kind="ExternalInput") for l in range(L)]
    wroot_d = [nc.dram_tensor(f"wroot{l}", [dims[l], dims[l + 1]], f32, kind="ExternalInput") for l in range(L)]
    brel_d = [nc.dram_tensor(f"brel{l}", [dims[l + 1], 1], f32, kind="ExternalInput") for l in range(L)]
    out_d = nc.dram_tensor("out", [NPC, dims[L]], f32, kind="ExternalOutput")
    dbg = {}
    if debug:
        for l in range(L):
            dbg[f"agg{l}"] = nc.dram_tensor(f"dbg_agg{l}", [NB * 128, dims[l + 1]],
                                            f32, kind="ExternalOutput")

    groups = [list(range(ncores))]
    # last block whose rows complete ybounce chunk q
    ag_block = [(CPC * (q + 1) + 127) // 128 - 1 for q in range(NCHUNK)]

    with tile.TileContext(nc) as tc:
        with (
            tc.tile_pool(name="const", bufs=1) as cpool,
            tc.tile_pool(name="edge", bufs=1) as epool,
            tc.tile_pool(name="xts", bufs=1) as xpool,
            tc.tile_pool(name="idxp", bufs=6) as ipool,
            tc.tile_pool(name="gath", bufs=6) as gpool,
            tc.tile_pool(name="msg", bufs=4) as mpool,
            tc.tile_pool(name="ohp", bufs=4) as opool,
            tc.tile_pool(name="small", bufs=4) as spool,
            tc.tile_pool(name="agg", bufs=1) as apool,
            tc.tile_pool(name="psum", bufs=1, space="PSUM") as ppool,
            tc.tile_pool(name="dram", bufs=1, space="DRAM") as dpool,
        ):
            # ---- load constants / edge data ----
            dstl_sb = epool.tile([128, NCHUNK * NB * T_chunk], f16)
            nc.sync.dma_start(dstl_sb[:], dstl_d.ap()[:])
            wgt_sb = epool.tile([128, NCHUNK * NB * T_chunk], f32)
            nc.sync.dma_start(wgt_sb[:], wgt_d.ap()[:])
            iota_sb = cpool.tile([128, 128], f16)
            nc.sync.dma_start(iota_sb[:], iota_d.ap()[:])
            ident_sb = cpool.tile([128, 128], f32)
            nc.sync.dma_start(ident_sb[:], ident_d.ap()[:])
            wrel_sb, wroot_sb, brel_sb = [], [], []
            for l in range(L):
                t1 = cpool.tile([dims[l], dims[l + 1]], f32, name=f"wrel_sb{l}")
                nc.sync.dma_start(t1[:], wrel_d[l].ap()[:])
                wrel_sb.append(t1)
                t2 = cpool.tile([dims[l], dims[l + 1]], f32, name=f"wroot_sb{l}")
                nc.sync.dma_start(t2[:], wroot_d[l].ap()[:])
                wroot_sb.append(t2)
                t3 = cpool.tile([dims[l + 1], 1], f32, name=f"brel_sb{l}")
                nc.sync.dma_start(t3[:], brel_d[l].ap()[:])
                brel_sb.append(t3)

            xT_cur = xpool.tile([dims[0], NPC], f32, name="xT_l0", tag="xT", bufs=2)
            nc.sync.dma_start(xT_cur[:], xT0_d.ap()[:])

            # chunk tables for the current layer: list of APs [CH, PAD]
            tables = [x64_d.ap()[q * CH:(q + 1) * CH, :] for q in range(NCHUNK)]

            for l in range(L):
                c = dims[l + 1]
                din = dims[l]
                agg_all = apool.tile([128, NB * cmax], f32, name=f"agg_all{l}",
                                     tag="aggall", bufs=2)
                if l < L - 1:
                    c2 = dims[l + 2]
                    xT_next = xpool.tile([c, NPC], f32, name=f"xT_l{l + 1}",
                                         tag="xT", bufs=2)
                    ybounce = [dpool.tile([CPC, c2], f32, name=f"yb{l + 1}_{q}")
                               for q in range(NCHUNK)]
                    ytabc = [dpool.tile([CH, c2], f32, name=f"ytc{l + 1}_{q}")
                             for q in range(NCHUNK)]
                    ytab64 = [dpool.tile([CH, PAD], f32, name=f"yt64_{l + 1}_{q}")
                              for q in range(NCHUNK)]
                else:
                    xT_next = None

                for q in range(NCHUNK):
                    for b in range(NB):
                        g = q * NB + b
                        es = slice(g * T_chunk, (g + 1) * T_chunk)
                        gmax = int(gcounts[q][b])
                        idx_sb = ipool.tile([128, IC], i16, name=f"idx{l}_{g}",
                                            tag="idx")
                        nc.sync.dma_start(idx_sb[:], idx_d.ap()[g])
                        gth = gpool.tile([128, T_chunk * PAD], f32,
                                         name=f"gth{l}_{g}", tag="gth")
                        if l == 0 and q == 0 and b < 6:
                            # first-touch the ring bufs: slots skipped by a
                            # short num_idxs_reg must hold finite data
                            nc.gpsimd.memset(gth[:], 0)
                        t0 = 0
                        while t0 < T_chunk:
                            tn = min(8, T_chunk - t0)
                            reg = max(0, min(gmax - t0 * 128, tn * 128))
                            if reg == 0:
                                break
                            nc.gpsimd.dma_gather(
                                out_ap=gth[:, t0 * PAD:(t0 + tn) * PAD]
                                    .rearrange("p (t e) -> p t e", e=PAD),
                                in_ap=tables[q][:],
                                idxs_ap=idx_sb[:, t0 * 8:(t0 + tn) * 8],
                                num_idxs=tn * 128,
                                num_idxs_reg=reg,
                                elem_size=PAD,
                                queue_num=qctr[0] % NQ,
                            )
                            qctr[0] += 1
                            t0 += tn
                        msg = mpool.tile([128, T_chunk * c], f16,
                                         name=f"msg{l}_{g}", tag="msg")
                        nc.vector.tensor_tensor(
                            out=msg[:].rearrange("p (t d) -> p t d", d=c),
                            in0=gth[:].rearrange("p (t e) -> p t e", e=PAD)[:, :, :c],
                            in1=wgt_sb[:, es].to_broadcast([128, T_chunk, c]),
                            op=mybir.AluOpType.mult,
                        )
                        oh = opool.tile([128, T_chunk * 128], f16,
                                        name=f"oh{l}_{g}", tag="oh")
                        nc.vector.tensor_tensor(
                            out=oh[:].rearrange("p (t n) -> p t n", n=128),
                            in0=dstl_sb[:, es].to_broadcast([128, T_chunk, 128]),
                            in1=iota_sb[:, None, :].to_broadcast([128, T_chunk, 128]),
                            op=mybir.AluOpType.is_equal,
                        )
                        aggps = ppool.tile([128, c], f32, name=f"aggps{l}_{g}",
                                           tag="agg", bufs=2)
                        for t in range(T_chunk):
                            nc.tensor.matmul(
                                out=aggps[:],
                                lhsT=oh[:, t * 128:(t + 1) * 128],
                                rhs=msg[:, t * c:(t + 1) * c],
                                start=(t == 0), stop=(t == T_chunk - 1),
                            )
                        asl = agg_all[:, b * cmax:b * cmax + c]
                        if q == 0:
                            nc.vector.tensor_copy(asl, aggps[:])
                        else:
                            nc.vector.tensor_add(out=asl, in0=asl, in1=aggps[:])

                        if q < NCHUNK - 1:
                            continue
                        # ---- epilogue for block b (all chunks accumulated) --
                        cnt = 128 if b < NB - 1 else last_cnt
                        if debug:
                            nc.sync.dma_start(
                                dbg[f"agg{l}"].ap()[b * 128:(b + 1) * 128, :], asl)
                        aggT = ppool.tile([c, 128], f32, name=f"aggT{l}_{b}",
                                          tag="aggT", bufs=2)
                        nc.tensor.transpose(out=aggT[:], in_=asl,
                                            identity=ident_sb[:])
                        zps = ppool.tile([c, 128], f32, name=f"zps{l}_{b}",
                                         tag="z", bufs=2)
                        nc.tensor.matmul(
                            out=zps[:, :cnt],
                            lhsT=wroot_sb[l][:],
                            rhs=xT_cur[:, b * 128:b * 128 + cnt],
                            start=True, stop=True,
                        )
                        aggT_sb = spool.tile([c, 128], f32,
                                             name=f"aggT_sb{l}_{b}", tag="aggTsb")
                        nc.vector.tensor_copy(aggT_sb[:], aggT[:])
                        z_sb = spool.tile([c, 128], f32, name=f"z_sb{l}_{b}",
                                          tag="zsb")
                        nc.vector.tensor_add(out=z_sb[:, :cnt], in0=zps[:, :cnt],
                                             in1=aggT_sb[:, :cnt])
                        if l < L - 1:
                            nc.scalar.activation(
                                out=xT_next[:, b * 128:b * 128 + cnt],
                                in_=z_sb[:, :cnt],
                                func=mybir.ActivationFunctionType.Relu,
                                bias=brel_sb[l][:],
                            )
                            yps = ppool.tile([128, c2], f32, name=f"yps{l}_{b}",
                                             tag="y", bufs=2)
                            nc.tensor.matmul(
                                out=yps[:cnt, :],
                                lhsT=xT_next[:, b * 128:b * 128 + cnt],
                                rhs=wrel_sb[l + 1][:],
                                start=True, stop=True,
                            )
                            ysb = spool.tile([128, c2], f32, name=f"ysb{l}_{b}",
                                             tag="ysb")
                            nc.vector.tensor_copy(ysb[:cnt, :], yps[:cnt, :])
                            r0 = b * 128
                            r1 = r0 + cnt
                            for qq in range(NCHUNK):
                                s0, s1 = qq * CPC, (qq + 1) * CPC
                                a0, a1 = max(r0, s0), min(r1, s1)
                                if a0 < a1:
                                    nc.sync.dma_start(
                                        ybounce[qq][a0 - s0:a1 - s0, :],
                                        ysb[a0 - r0:a1 - r0, :])
                            for qq in range(NCHUNK):
                                if b == ag_block[qq]:
                                    nc.gpsimd.collective_compute(
                                        "AllGather", mybir.AluOpType.bypass,
                                        replica_groups=groups,
                                        ins=[ybounce[qq][:].opt()],
                                        outs=[ytabc[qq][:].opt()],
                                    )
                                    nc.sync.dma_start(ytab64[qq][:, :c2],
                                                      ytabc[qq][:])
                        else:
                            r_sb = spool.tile([c, 128], f32, name=f"r_sb{b}",
                                              tag="rsb")
                            nc.scalar.activation(
                                out=r_sb[:, :cnt], in_=z_sb[:, :cnt],
                                func=mybir.ActivationFunctionType.Relu,
                                bias=brel_sb[l][:],
                            )
                            tps = ppool.tile([128, c], f32, name=f"tps{b}",
                                             tag="y", bufs=2)
                            nc.tensor.transpose(out=tps[:cnt, :], in_=r_sb[:, :cnt],
                                                identity=ident_sb[:c, :c])
                            zb = spool.tile([128, c], f32, name=f"zb{b}", tag="zb")
                            nc.vector.tensor_copy(zb[:cnt, :], tps[:cnt, :])
                            mx = spool.tile([128, 1], f32, name=f"mx{b}", tag="mx")
                            nc.vector.tensor_tensor(
                                out=mx[:cnt, :], in0=zb[:cnt, 0:1], in1=zb[:cnt, 1:2],
                                op=mybir.AluOpType.max,
                            )
                            zs = spool.tile([128, c], f32, name=f"zs{b}", tag="zs")
                            nc.vector.tensor_tensor(
                                out=zs[:cnt, :], in0=zb[:cnt, :],
                                in1=mx[:cnt, :].to_broadcast([cnt, c]),
                                op=mybir.AluOpType.subtract,
                            )
                            esb = spool.tile([128, c], f32, name=f"esb{b}",
                                             tag="esb")
                            nc.scalar.activation(
                                out=esb[:cnt, :], in_=zs[:cnt, :],
                                func=mybir.ActivationFunctionType.Exp,
                            )
                            ssb = spool.tile([128, 1], f32, name=f"ssb{b}",
                                             tag="ssb")
                            nc.vector.tensor_add(out=ssb[:cnt, :],
                                                 in0=esb[:cnt, 0:1],
                                                 in1=esb[:cnt, 1:2])
                            rcp = spool.tile([128, 1], f32, name=f"rcp{b}",
                                             tag="rcp")
                            nc.vector.reciprocal(rcp[:cnt, :], ssb[:cnt, :])
                            osb = spool.tile([128, c], f32, name=f"osb{b}",
                                             tag="osb")
                            nc.vector.tensor_tensor(
                                out=osb[:cnt, :], in0=esb[:cnt, :],
                                in1=rcp[:cnt, :].to_broadcast([cnt, c]),
                                op=mybir.AluOpType.mult,
                            )
                            nc.sync.dma_start(
                                out_d.ap()[b * 128:b * 128 + cnt, :], osb[:cnt, :])
                if l < L - 1:
                    tables = [ytab64[q] for q in range(NCHUNK)]
                    xT_cur = xT_next


def make_host_inputs(inputs, N, dims, ncores):
    x = np.asarray(inputs["x"], np.float32)
    (idx16, dstl, wgt, T_chunk, NB, NPC, last_cnt, CPC, CH, gcounts) = preprocess(
        inputs["edge_index"], inputs["edge_weight"], N, ncores)
    iota = np.broadcast_to(np.arange(128, dtype=np.float16), (128, 128)).copy()
    ident = np.eye(128, dtype=np.float32)
    # layer-0 gather tables: y0 = x @ W_rel0, rows permuted chunk-major
    # (chunk q, core k, row r) <- global node k*NPC + q*CPC + r
    y0 = (x @ np.asarray(inputs["w_rel0"], np.float32)).astype(np.float32)
    x64 = np.zeros((N, PAD), np.float32)
    n = np.arange(N)
    m = (n % NPC) // CPC * CH + (n // NPC) * CPC + (n % NPC) % CPC
    x64[m, :dims[1]] = y0
    L = len(dims) - 1
    common = {"iota": iota, "ident": ident, "x64": x64}
    for l in range(L):
        common[f"wrel{l}"] = np.asarray(inputs[f"w_rel{l}"], np.float32)
        common[f"wroot{l}"] = np.asarray(inputs[f"w_root{l}"], np.float32)
        common[f"brel{l}"] = np.asarray(inputs[f"b_rel{l}"], np.float32).reshape(-1, 1)
    in_maps = []
    for k in range(ncores):
        mm = dict(common)
        mm["idx16"] = idx16[k]
        mm["dstl"] = dstl[k]
        mm["wgt"] = wgt[k]
        mm["xT0"] = np.ascontiguousarray(x[k * NPC:(k + 1) * NPC].T)
        in_maps.append(mm)
    return in_maps, T_chunk, NB, NPC, last_cnt, CPC, CH, gcounts


def _install_loud_hook():
    import traceback
    from concourse import bass2jax
    bass2jax.install_neuronx_cc_hook()
    try:
        import libneuronxla
    except ImportError:
        return
    hook = libneuronxla.neuronx_cc
    def loud(*a, **k):
        try:
            return hook(*a, **k)
        except BaseException:
            traceback.print_exc()
            raise
    libneuronxla.neuronx_cc = loud
    bass2jax.install_neuronx_cc_hook = lambda: None


def build_all(inputs, N, dims, ncores, debug=False):
    _install_loud_hook()
    in_maps, T_chunk, NB, NPC, last_cnt, CPC, CH, gcounts = make_host_inputs(
        inputs, N, dims, ncores)
    nc = bacc.Bacc("TRN2", target_bir_lowering=False, debug=False,
                   num_devices=ncores, num_swdge_queues=4)
    build_gnn(nc, N, NPC, NB, T_chunk, dims, ncores, last_cnt, CPC, CH,
              gcounts, debug=debug)
    nc.compile()
    return nc, in_maps


def run_gnn(inputs, N, dims, ncores=8, trace=False, debug=False):
    from concourse.bass_utils import run_bass_kernel_spmd
    nc, in_maps = build_all(inputs, N, dims, ncores, debug=debug)
    res = run_bass_kernel_spmd(nc, in_maps, core_ids=list(range(ncores)),
                               trace=trace)
    out = np.concatenate([res.results[k]["out"] for k in range(ncores)], axis=0)
    return out, res


def kernel(**inputs):
    out, _res = run_gnn(inputs, N_NODES, DIMS, ncores=N_CORES, trace=False)
    return out


def kernel_traced(**inputs):
    """Like kernel() but also returns the BassKernelResults (exec_time_ns etc)."""
    return run_gnn(inputs, N_NODES, DIMS, ncores=N_CORES, trace=True)
